# revision 1
# baseline (speedup 1.0000x reference)
"""LoRADense (per-token adapter routing) Bass kernel for 8 Trainium2 NeuronCores.

Math (reference):
    base  = x @ kernel + bias                      # (N, F)
    a     = lora_a[adapter_ids]                    # (N, D, R) gather
    b     = lora_b[adapter_ids]                    # (N, R, F) gather
    lr    = einsum('nd,ndr->nr', x, a)             # (N, R)
    delta = einsum('nr,nrf->nf', lr, b)            # (N, F)
    out   = base + delta

Strategy:
  - Data parallel over tokens: 1024 tokens per core, all weights replicated.
  - Host sorts each core's tokens by adapter id (stable argsort). After
    sorting, each 128-token block's adapters fit inside a static window of
    W consecutive 128-row slabs of the concatenated LoRA matrices
    A_cat = lora_a.transpose(1,0,2).reshape(D, S*R)  (D, 1024)
    B_stk = lora_b.reshape(S*R, F)                   (1024, F)
    The window start slab sigma_b = clamp(b - W//2, 0, 8 - W) is the same for
    every core (SPMD-safe).  Containment is verified on the host; if it ever
    fails, W is widened (W=8 degenerates to the fully dense masked form,
    which is always correct).
  - Device per 128-token block b:
      stage A: lrT[sr_window, tok] = A_cat_slab^T-style matmuls (bf16),
               masked per (sr row, token) by is_equal(adapter_id, sr//16),
               result kept in SBUF as bf16.
      stage B: one PSUM accumulation per (block, f-half):
               8x f32r matmuls  (base: xsT^T @ kernel_slab)
             + Wx bf16 matmuls  (delta: lrm^T @ B_slab)
               then +bias on DVE and DMA to DRAM.
  - Host un-permutes the rows of the result.
"""

import numpy as np
import ml_dtypes

import concourse.bacc as bacc
import concourse.bass as bass
import concourse.mybir as mybir
import concourse.tile as tile
from concourse.bass_utils import run_bass_kernel_spmd

# Problem constants (hardcoded per harness contract).
N = 8192          # tokens
D = 1024          # input dim
F = 1024          # output features
R = 16            # lora rank
S = 64            # adapter slots
SR = S * R        # 1024
NCORES = 8
NTOK = N // NCORES            # 1024 tokens per core
P = 128                       # partitions
NT = NTOK // P                # 8 token blocks per core
KD = D // P                   # 8 contraction slabs over D
SRS = SR // P                 # 8 slabs over S*R
FH = 2                        # f halves of 512
FHW = F // FH                 # 512

BF16 = ml_dtypes.bfloat16

# Toggles (test.py pokes these).
TRACE = False
LAST_RESULTS = None
LAST_IN_MAPS = None
LAST_NC = None
LAST_W = None
LAST_SIGMAS = None
VARIANT = 3  # 1=base-only(f32r), 2=lora-only(bf16), 3=full
REPS = 1     # emit the whole compute this many times (benchmarking only)

_NC_CACHE = {}


def _sigmas_for(w):
    return tuple(min(max(b - w // 2, 0), SRS - w) for b in range(NT))


def _build_nc(w, sigmas):
    """Build the single-core Bass program (same program runs on all 8 cores)."""
    f32 = mybir.dt.float32
    f32r = mybir.dt.float32r
    bf16 = mybir.dt.bfloat16

    nc = bacc.Bacc("TRN2", target_bir_lowering=False, debug=False)

    # DRAM I/O. Layouts are pre-shuffled on the host so every DMA is a plain
    # contiguous [partition, free...] copy.
    xt = nc.dram_tensor("xt", [P, KD, NTOK], f32r, kind="ExternalInput")   # xsT: [p, d_o, tok]
    xtb = nc.dram_tensor("xtb", [P, KD, NTOK], bf16, kind="ExternalInput")  # xsT in bf16
    wk = nc.dram_tensor("wk", [P, KD, F], f32r, kind="ExternalInput")      # kernel: [p, d_o, f]
    ac = nc.dram_tensor("ac", [P, KD, SR], bf16, kind="ExternalInput")     # A_cat: [p, d_o, sr]
    bs = nc.dram_tensor("bs", [P, SRS, F], bf16, kind="ExternalInput")     # B_stk: [p, sr_o, f]
    msk = nc.dram_tensor("msk", [P, NT * w, P], f32, kind="ExternalInput")  # host masks
    bib = nc.dram_tensor("bib", [P, F], f32, kind="ExternalInput")         # bias bcast
    out_s = nc.dram_tensor("out_s", [NTOK, F], f32, kind="ExternalOutput")

    with tile.TileContext(nc) as tc:
        with (
            tc.tile_pool(name="const", bufs=1) as cpool,
            tc.tile_pool(name="work", bufs=4) as wpool,
            tc.tile_pool(name="lrps", bufs=4, space="PSUM") as lrps,
            tc.tile_pool(name="outps", bufs=4, space="PSUM") as outps,
        ):
            msk_sb = cpool.tile([P, NT * w, P], f32)
            nc.sync.dma_start(msk_sb[:], msk[:])
            bib_sb = cpool.tile([P, F], f32)
            nc.sync.dma_start(bib_sb[:], bib[:])

            # Per-slab DMAs so compute on slab k starts as soon as it lands.
            xt_sb = cpool.tile([P, KD, NTOK], f32r)
            xtb_sb = cpool.tile([P, KD, NTOK], bf16)
            wk_sb = cpool.tile([P, KD, F], f32r)
            ac_sb = cpool.tile([P, KD, SR], bf16)
            bs_sb = cpool.tile([P, SRS, F], bf16)
            for k in range(KD):
                nc.sync.dma_start(xtb_sb[:, k], xtb[:, k])
                nc.sync.dma_start(ac_sb[:, k], ac[:, k])
                nc.sync.dma_start(xt_sb[:, k], xt[:, k])
                nc.sync.dma_start(wk_sb[:, k], wk[:, k])
                nc.sync.dma_start(bs_sb[:, k], bs[:, k])

            # Masked low-rank activations, bf16: [sr_p, b*w + j, tok]
            lrm_sb = cpool.tile([P, NT * w, P], bf16)

            for b in [bb for _ in range(REPS) for bb in range(NT)]:
                sig = sigmas[b]
                tok = slice(b * P, (b + 1) * P)

                # ---- stage A: lrT window slabs + mask ----
                for j in range(w if VARIANT != 1 else 0):
                    o = sig + j
                    ps = lrps.tile([P, P], mybir.dt.float32, tag="lr")
                    if VARIANT == 5:
                        nc.vector.memset(ps[:], 0.0)
                    else:
                        for k in range(KD):
                            nc.tensor.matmul(
                                ps[:],
                                ac_sb[:, k, o * P:(o + 1) * P],
                                xtb_sb[:, k, tok],
                                start=(k == 0),
                                stop=(k == KD - 1),
                            )
                    if VARIANT == 4:
                        nc.vector.tensor_copy(out=lrm_sb[:, b * w + j], in_=ps[:])
                    else:
                        # msk[p, b*w+j, t] = (ids[t] == (o*128+p)//16), host-built
                        nc.vector.tensor_tensor(
                            lrm_sb[:, b * w + j],
                            ps[:],
                            msk_sb[:, b * w + j],
                            mybir.AluOpType.mult,
                        )

                # ---- stage B: fused base + delta accumulation ----
                for h in range(FH):
                    fs = slice(h * FHW, (h + 1) * FHW)
                    po = outps.tile([P, FHW], mybir.dt.float32, tag="out")
                    if VARIANT != 2:
                        for k in range(KD):
                            nc.tensor.matmul(
                                po[:],
                                xt_sb[:, k, tok],
                                wk_sb[:, k, fs],
                                start=(k == 0),
                                stop=(VARIANT == 1 and k == KD - 1),
                            )
                    if VARIANT != 1:
                        for j in range(w):
                            o = sig + j
                            nc.tensor.matmul(
                                po[:],
                                lrm_sb[:, b * w + j],
                                bs_sb[:, o, fs],
                                start=(VARIANT == 2 and j == 0),
                                stop=(j == w - 1),
                            )
                    ob = wpool.tile([P, FHW], mybir.dt.float32, tag="ob")
                    nc.any.tensor_tensor(
                        ob[:], po[:], bib_sb[:, fs], mybir.AluOpType.add
                    )
                    nc.sync.dma_start(out_s[tok, fs], ob[:])

    nc.compile()
    return nc


def _get_nc(w, sigmas):
    key = (w, sigmas, VARIANT, REPS)
    if key not in _NC_CACHE:
        _NC_CACHE[key] = _build_nc(w, sigmas)
    return _NC_CACHE[key]


def kernel(x, adapter_ids, kernel, bias, lora_a, lora_b):
    global LAST_RESULTS
    x = np.ascontiguousarray(np.asarray(x, dtype=np.float32))
    adapter_ids = np.asarray(adapter_ids)
    kernel_w = np.ascontiguousarray(np.asarray(kernel, dtype=np.float32))
    bias = np.asarray(bias, dtype=np.float32)
    lora_a = np.asarray(lora_a, dtype=np.float32)
    lora_b = np.asarray(lora_b, dtype=np.float32)
    ids = adapter_ids.astype(np.int64)

    # Replicated weight layouts: [p, slab, free] with contiguous per-partition runs.
    a_cat = lora_a.transpose(1, 0, 2).reshape(D, SR)                  # (D, S*R)
    b_stk = lora_b.reshape(SR, F)                                     # (S*R, F)
    wk_l = np.ascontiguousarray(kernel_w.reshape(KD, P, F).transpose(1, 0, 2))
    ac_l = np.ascontiguousarray(
        a_cat.reshape(KD, P, SR).transpose(1, 0, 2).astype(BF16))
    bs_l = np.ascontiguousarray(
        b_stk.reshape(SRS, P, F).transpose(1, 0, 2).astype(BF16))
    bib_l = np.ascontiguousarray(np.broadcast_to(bias, (P, F)))

    # Per-core shards: sort tokens by adapter id.
    perms, ids_s_all = [], []
    for c in range(NCORES):
        lo = c * NTOK
        sh_ids = ids[lo:lo + NTOK]
        perm = np.argsort(sh_ids, kind="stable")
        perms.append(perm)
        ids_s_all.append(sh_ids[perm])

    # Pick the narrowest static window W whose containment holds on all cores.
    w_pick = None
    for w in (3, 4, 6, 8):
        sigmas = _sigmas_for(w)
        ok = True
        for ids_s in ids_s_all:
            for b in range(NT):
                blk = ids_s[b * P:(b + 1) * P]
                lo_a, hi_a = sigmas[b] * 8, (sigmas[b] + w) * 8
                if blk.min() < lo_a or blk.max() >= hi_a:
                    ok = False
                    break
            if not ok:
                break
        if ok:
            w_pick = w
            break
    assert w_pick is not None
    sigmas = _sigmas_for(w_pick)

    # Per-(slab-row, window-slab) adapter index: adiv[p, o] = (o*128+p)//16
    adiv = (np.arange(SRS)[None, :] * P + np.arange(P)[:, None]) // R  # (P, SRS)
    in_maps = []
    for c in range(NCORES):
        lo = c * NTOK
        ids_s = ids_s_all[c]
        xs = x[lo:lo + NTOK][perms[c]]                                # (NTOK, D)
        xt_l = np.ascontiguousarray(
            xs.T.reshape(KD, P, NTOK).transpose(1, 0, 2))             # (P, KD, NTOK)
        # msk[p, b*w+j, t] = (ids_s[b*128+t] == (sigma_b+j)*8 + p//16)
        slabs = np.array([sigmas[b] + j for b in range(NT)
                          for j in range(w_pick)])                    # (NT*w,)
        ids_blk = ids_s.reshape(NT, P)                                # (NT, P)
        ids_rep = np.repeat(ids_blk, w_pick, axis=0)                  # (NT*w, P)
        msk_l = np.ascontiguousarray(
            (adiv[:, slabs][:, :, None] == ids_rep[None, :, :])
            .astype(np.float32))                                      # (P, NT*w, P)
        in_maps.append({
            "xt": xt_l, "xtb": xt_l.astype(BF16), "wk": wk_l, "ac": ac_l,
            "bs": bs_l, "msk": msk_l, "bib": bib_l,
        })

    nc = _get_nc(w_pick, sigmas)
    res = run_bass_kernel_spmd(nc, in_maps, core_ids=list(range(NCORES)),
                               trace=TRACE)
    global LAST_IN_MAPS, LAST_NC, LAST_W, LAST_SIGMAS
    LAST_RESULTS = res
    LAST_IN_MAPS = in_maps
    LAST_NC = nc
    LAST_W = w_pick
    LAST_SIGMAS = sigmas

    out = np.empty((N, F), dtype=np.float32)
    for c in range(NCORES):
        seg = out[c * NTOK:(c + 1) * NTOK]
        seg[perms[c]] = res.results[c]["out_s"]
    return out



# revision 2
# speedup vs baseline: 1.6699x; 1.6699x over previous
"""LoRADense (per-token adapter routing) Bass kernel for 8 Trainium2 NeuronCores.

Math (reference):
    base  = x @ kernel + bias                      # (N, F)
    a     = lora_a[adapter_ids]                    # (N, D, R) gather
    b     = lora_b[adapter_ids]                    # (N, R, F) gather
    lr    = einsum('nd,ndr->nr', x, a)             # (N, R)
    delta = einsum('nr,nrf->nf', lr, b)            # (N, F)
    out   = base + delta

Strategy (v2):
  - GLOBAL sort of all 8192 tokens by adapter id on the host; core c gets the
    contiguous sorted run [1024c, 1024(c+1)).  Each core therefore only sees a
    narrow band of ~9-10 consecutive adapter ids.  The host gathers, per core,
    just the NSLAB*128 rows (NSLAB*8 adapters, normally NSLAB=2) of the
    concatenated LoRA factors that this core needs, re-based so the device
    program is identical on every core (SPMD-safe):
       A_loc = lora_a.T-cat columns [a0*R, (a0+8*NSLAB)*R)   (D, NSLAB*128)
       B_loc = lora_b-stack rows    [a0*R, (a0+8*NSLAB)*R)   (NSLAB*128, F)
  - Everything runs in bf16 (f32 PSUM accumulation), output stored bf16.
  - Transposed compute: out^T[f, tok] so the moving operand is always the
    token axis (512-wide chunks) and every stationary weight block streams
    many tokens:
      stage A: lr[sr_loc, tok]  = A_loc^T @ x   (accumulate over 8 D-slabs),
               masked per (sr row, token) on DVE -> bf16 lrm in SBUF.
      stage B: po[f_blk, tok]   = sum_k Wk^T @ x  +  sum_o B_o^T @ lrm_o
               (one PSUM group of 8+NSLAB matmuls), then +bias (per-partition
               scalar) fused with the f32->bf16 convert, DMA to DRAM.
  - Host un-permutes rows and upcasts to f32.
"""

import numpy as np
import ml_dtypes

import concourse.bacc as bacc
import concourse.bass as bass
import concourse.mybir as mybir
import concourse.tile as tile
from concourse.bass_utils import run_bass_kernel_spmd

# Problem constants (hardcoded per harness contract).
N = 8192          # tokens
D = 1024          # input dim
F = 1024          # output features
R = 16            # lora rank
S = 64            # adapter slots
SR = S * R        # 1024
NCORES = 8
NTOK = N // NCORES            # 1024 tokens per core
P = 128                       # partitions
NT = NTOK // P                # 8 token blocks per core
KD = D // P                   # 8 contraction slabs over D
TCH = 512                     # moving-operand token chunk
NCH = NTOK // TCH             # 2 chunks per core

BF16 = ml_dtypes.bfloat16

# Toggles (test.py pokes these).
TRACE = False
LAST_RESULTS = None
LAST_IN_MAPS = None
LAST_NC = None
LAST_NS = None

_NC_CACHE = {}


def _build_nc(ns):
    """Build the single-core Bass program (same program runs on all 8 cores)."""
    f32 = mybir.dt.float32
    bf16 = mybir.dt.bfloat16

    nc = bacc.Bacc("TRN2", target_bir_lowering=False, debug=False)

    # DRAM I/O. Layouts are pre-shuffled on the host so every DMA is a plain
    # contiguous [partition, free...] copy.
    xtb = nc.dram_tensor("xtb", [P, KD, NTOK], bf16, kind="ExternalInput")   # x^T: [d_p, k, tok]
    wkb = nc.dram_tensor("wkb", [P, KD, F], bf16, kind="ExternalInput")      # kernel: [d_p, k, f]
    ac = nc.dram_tensor("ac", [P, KD, ns * P], bf16, kind="ExternalInput")   # A_loc: [d_p, k, sr_loc]
    bs = nc.dram_tensor("bs", [P, ns, F], bf16, kind="ExternalInput")        # B_loc: [sr_p, o, f]
    msk = nc.dram_tensor("msk", [P, ns, NTOK], bf16, kind="ExternalInput")   # host masks
    bia = nc.dram_tensor("bia", [P, KD], f32, kind="ExternalInput")          # bias: [f_p, j]
    out_s = nc.dram_tensor("out_s", [KD, P, NTOK], bf16, kind="ExternalOutput")

    with tile.TileContext(nc) as tc:
        with (
            tc.tile_pool(name="const", bufs=1) as cpool,
            tc.tile_pool(name="work", bufs=4) as wpool,
            tc.tile_pool(name="lrps", bufs=2, space="PSUM") as lrps,
            tc.tile_pool(name="outps", bufs=4, space="PSUM") as outps,
        ):
            ac_sb = cpool.tile([P, KD, ns * P], bf16)
            nc.sync.dma_start(ac_sb[:], ac[:])
            msk_sb = cpool.tile([P, ns, NTOK], bf16)
            nc.sync.dma_start(msk_sb[:], msk[:])
            # Per-slab DMAs so compute on slab k starts as soon as it lands.
            xtb_sb = cpool.tile([P, KD, NTOK], bf16)
            for k in range(KD):
                nc.sync.dma_start(xtb_sb[:, k], xtb[:, k])
            bs_sb = cpool.tile([P, ns, F], bf16)
            nc.sync.dma_start(bs_sb[:], bs[:])
            wkb_sb = cpool.tile([P, KD, F], bf16)
            for k in range(KD):
                nc.sync.dma_start(wkb_sb[:, k], wkb[:, k])
            bia_sb = cpool.tile([P, KD], f32)
            nc.sync.dma_start(bia_sb[:], bia[:])

            # Masked low-rank activations, bf16: [sr_p, o, tok]
            lrm_sb = cpool.tile([P, ns, NTOK], bf16)

            # ---- stage A: lr window slabs + mask ----
            for o in range(ns):
                for t in range(NCH):
                    tok = slice(t * TCH, (t + 1) * TCH)
                    ps = lrps.tile([P, TCH], mybir.dt.float32, tag="lr")
                    for k in range(KD):
                        nc.tensor.matmul(
                            ps[:],
                            ac_sb[:, k, o * P:(o + 1) * P],
                            xtb_sb[:, k, tok],
                            start=(k == 0),
                            stop=(k == KD - 1),
                        )
                    # msk[p, o, t] = (lid[t] == (o*128+p)//16), host-built
                    nc.vector.tensor_tensor(
                        lrm_sb[:, o, tok],
                        ps[:],
                        msk_sb[:, o, tok],
                        mybir.AluOpType.mult,
                    )

            # ---- stage B: fused base + delta accumulation, transposed ----
            for j in range(KD):
                fb = slice(j * P, (j + 1) * P)
                for t in range(NCH):
                    tok = slice(t * TCH, (t + 1) * TCH)
                    po = outps.tile([P, TCH], mybir.dt.float32, tag="out")
                    for k in range(KD):
                        nc.tensor.matmul(
                            po[:],
                            wkb_sb[:, k, fb],
                            xtb_sb[:, k, tok],
                            start=(k == 0),
                            stop=False,
                        )
                    for o in range(ns):
                        nc.tensor.matmul(
                            po[:],
                            bs_sb[:, o, fb],
                            lrm_sb[:, o, tok],
                            start=False,
                            stop=(o == ns - 1),
                        )
                    ob = wpool.tile([P, TCH], bf16, tag="ob")
                    nc.any.tensor_scalar_add(ob[:], po[:], bia_sb[:, j:j + 1])
                    nc.sync.dma_start(out_s[j, :, tok], ob[:])

    nc.compile()
    return nc


def _get_nc(ns):
    if ns not in _NC_CACHE:
        _NC_CACHE[ns] = _build_nc(ns)
    return _NC_CACHE[ns]


def kernel(x, adapter_ids, kernel, bias, lora_a, lora_b):
    global LAST_RESULTS, LAST_IN_MAPS, LAST_NC, LAST_NS
    x = np.ascontiguousarray(np.asarray(x, dtype=np.float32))
    adapter_ids = np.asarray(adapter_ids)
    kernel_w = np.asarray(kernel, dtype=np.float32)
    bias = np.asarray(bias, dtype=np.float32)
    lora_a = np.asarray(lora_a, dtype=np.float32)
    lora_b = np.asarray(lora_b, dtype=np.float32)
    ids = adapter_ids.astype(np.int64)

    # Global stable sort by adapter id; each core gets a contiguous run.
    perm = np.argsort(ids, kind="stable")
    ids_s = ids[perm]
    xs_all = x[perm]

    # Per-core adapter band [a0_c, a0_c + 8*ns).
    a0s, spans = [], []
    for c in range(NCORES):
        blk = ids_s[c * NTOK:(c + 1) * NTOK]
        a0s.append(int(blk.min()))
        spans.append(int(blk.max()) - int(blk.min()) + 1)
    ns = max(2, int(np.ceil(max(spans) / 8)))
    a0s = [min(a0, S - 8 * ns) if 8 * ns < S else 0 for a0 in a0s]

    # Replicated weight layouts: [p, slab, free] with contiguous runs.
    a_cat = lora_a.transpose(1, 0, 2).reshape(D, SR)                  # (D, S*R)
    b_stk = lora_b.reshape(SR, F)                                     # (S*R, F)
    wk_l = np.ascontiguousarray(
        kernel_w.reshape(KD, P, F).transpose(1, 0, 2).astype(BF16))
    bia_l = np.ascontiguousarray(bias.reshape(KD, P).T.astype(np.float32))

    # Per-(slab-row, slab) local adapter index: (o*128+p)//16
    adiv = (np.arange(ns)[None, :] * P + np.arange(P)[:, None]) // R  # (P, ns)

    in_maps = []
    for c in range(NCORES):
        lo = c * NTOK
        a0 = a0s[c]
        sr0 = a0 * R
        xs = xs_all[lo:lo + NTOK]                                     # (NTOK, D)
        xt_l = np.ascontiguousarray(
            xs.T.reshape(KD, P, NTOK).transpose(1, 0, 2).astype(BF16))
        ac_l = np.ascontiguousarray(
            a_cat[:, sr0:sr0 + ns * P].reshape(KD, P, ns * P)
            .transpose(1, 0, 2).astype(BF16))
        bs_l = np.ascontiguousarray(
            b_stk[sr0:sr0 + ns * P].reshape(ns, P, F)
            .transpose(1, 0, 2).astype(BF16))
        lid = (ids_s[lo:lo + NTOK] - a0)                              # (NTOK,)
        msk_l = np.ascontiguousarray(
            (adiv[:, :, None] == lid[None, None, :]).astype(BF16))    # (P, ns, NTOK)
        in_maps.append({
            "xtb": xt_l, "wkb": wk_l, "ac": ac_l, "bs": bs_l,
            "msk": msk_l, "bia": bia_l,
        })

    nc = _get_nc(ns)
    res = run_bass_kernel_spmd(nc, in_maps, core_ids=list(range(NCORES)),
                               trace=TRACE)
    LAST_RESULTS = res
    LAST_IN_MAPS = in_maps
    LAST_NC = nc
    LAST_NS = ns

    out = np.empty((N, F), dtype=np.float32)
    for c in range(NCORES):
        # out_s[j, p, t] holds out^T for f = j*128+p -> reshape to (F, NTOK).
        core_out = res.results[c]["out_s"].reshape(F, NTOK).T
        out[perm[c * NTOK:(c + 1) * NTOK]] = core_out.astype(np.float32)
    return out


# revision 14
# speedup vs baseline: 2.1253x; 1.2727x over previous
"""LoRADense (per-token adapter routing) Bass kernel for 8 Trainium2 NeuronCores.

Math (reference):
    base  = x @ kernel + bias                      # (N, F)
    a     = lora_a[adapter_ids]                    # (N, D, R) gather
    b     = lora_b[adapter_ids]                    # (N, R, F) gather
    lr    = einsum('nd,ndr->nr', x, a)             # (N, R)
    delta = einsum('nr,nrf->nf', lr, b)            # (N, F)
    out   = base + delta

Strategy (v5):
  - GLOBAL sort of all 8192 tokens by adapter id on the host; core c gets the
    contiguous sorted run [1024c, 1024(c+1)).  Within a core, each 512-token
    chunk sees only ~5 consecutive adapter ids, so the host gathers, per
    (core, chunk), one 128-row band (8 adapters; spc slabs in general) of the
    concatenated LoRA factors, re-based so the device program is identical on
    every core (SPMD-safe).
  - Everything runs in bf16 (f32 PSUM accumulation), output stored bf16.
  - Transposed compute: out^T[f, tok] so the moving operand is always the
    token axis (512-wide chunks) and every stationary 128x128 block streams
    512 tokens:
      stage A: lr[sr_band, tok] = A_band^T @ x  (accumulate over 8 D-slabs),
               masked per (sr row, token) on DVE -> bf16 lrm in SBUF.
      stage B: po[f_blk, tok]   = sum_k Wk^T @ x  +  B_band^T @ lrm
               (one PSUM group of 8+spc matmuls), then +bias (per-partition
               scalar) fused with the f32->bf16 convert, DMA to DRAM.
  - k-major schedule in f-block passes sized to the 8 PSUM banks; pass 0
    carries stage A.  The per-k data (A band | x slab | first W f-blocks) is
    packed into ONE DMA per k so the stream feeds pass 0 just-in-time; the
    remaining W f-blocks stream during pass 1.
  - Host un-permutes rows and upcasts to f32.
"""

import numpy as np
import ml_dtypes

import concourse.bacc as bacc
import concourse.bass as bass
import concourse.mybir as mybir
import concourse.tile as tile
from concourse.bass_utils import run_bass_kernel_spmd

# Problem constants (hardcoded per harness contract).
N = 8192          # tokens
D = 1024          # input dim
F = 1024          # output features
R = 16            # lora rank
S = 64            # adapter slots
SR = S * R        # 1024
NCORES = 8
NTOK = N // NCORES            # 1024 tokens per core
P = 128                       # partitions
KD = D // P                   # 8 contraction slabs over D
TCH = 512                     # moving-operand token chunk
NCH = NTOK // TCH             # 2 chunks per core

BF16 = ml_dtypes.bfloat16

# Toggles (test.py pokes these).
TRACE = False
LAST_RESULTS = None
LAST_IN_MAPS = None
LAST_NC = None
LAST_NS = None

JUNK = 7
_NC_CACHE = {}


def _passes(spc):
    """f-block passes + whether stage A rides in pass 0, given PSUM budget 8."""
    n_lr = NCH * spc
    if n_lr <= 8 - NCH:  # room for at least one f-block next to the lr banks
        g0 = (8 - n_lr) // NCH
        jgs = [tuple(range(g0))]
        a_in_pass0 = True
    else:
        jgs = []
        a_in_pass0 = False
        g0 = 0
    j = g0
    while j < KD:
        g = min(8 // NCH, KD - 1 - j) if j < KD - 1 else 1
        g = max(1, min(g, KD - j - 1 if KD - j > 1 else 1))
        jgs.append(tuple(range(j, j + g)))
        j += g
    return jgs, a_in_pass0


def _build_nc(spc):
    """Build the single-core Bass program (same program runs on all 8 cores).

    spc = LoRA slabs (128-row bands) per 512-token chunk; normally 1.
    """
    f32 = mybir.dt.float32
    bf16 = mybir.dt.bfloat16
    nsl = NCH * spc                 # total gathered slabs per core
    jgs, a_in_p0 = _passes(spc)
    nja = len(jgs[0]) if a_in_p0 else 0   # f-blocks packed with the k-stream
    ACW = nsl * P                   # A-band columns in the pack
    XO = ACW                        # x offset in the pack
    WO = ACW + NTOK                 # W offset in the pack
    PKW = WO + nja * P              # pack width (bf16 elements)
    NJB = KD - nja                  # f-blocks in the second W stream

    nc = bacc.Bacc("TRN2", target_bir_lowering=False, debug=False)

    # DRAM I/O. Layouts are pre-packed on the host so every DMA is a plain
    # contiguous [partition, free...] copy.
    pk = nc.dram_tensor("pk", [P, KD, PKW], bf16, kind="ExternalInput")
    wkb = nc.dram_tensor("wkb", [P, KD, NJB * P], bf16, kind="ExternalInput")
    bs = nc.dram_tensor("bs", [P, nsl, F], bf16, kind="ExternalInput")
    msk = nc.dram_tensor("msk", [P, spc, NTOK], bf16, kind="ExternalInput")
    bia = nc.dram_tensor("bia", [P, KD], f32, kind="ExternalInput")
    out_s = nc.dram_tensor("out_s", [KD, P, NTOK], bf16, kind="ExternalOutput")

    with tile.TileContext(nc) as tc:
        with (
            tc.tile_pool(name="const", bufs=1) as cpool,
            tc.tile_pool(name="work", bufs=4) as wpool,
            tc.tile_pool(name="accp", bufs=8, space="PSUM") as accp,
        ):
            # Just-in-time DMA stream: one pack per D-slab k feeds pass 0.
            pk_sb = cpool.tile([P, KD, PKW], bf16)
            nc.sync.dma_start(pk_sb[:, 0, :XO + TCH], pk[:, 0, :XO + TCH])
            nc.sync.dma_start(pk_sb[:, 0, XO + TCH:], pk[:, 0, XO + TCH:])
            for k in range(1, KD):
                nc.sync.dma_start(pk_sb[:, k], pk[:, k])
            msk_sb = cpool.tile([P, spc, NTOK], bf16)
            nc.sync.dma_start(msk_sb[:], msk[:])
            bia_sb = cpool.tile([P, KD], f32)
            nc.sync.dma_start(bia_sb[:], bia[:])
            bs_sb = cpool.tile([P, nsl, F], bf16)
            nc.sync.dma_start(bs_sb[:], bs[:])
            wkb_sb = cpool.tile([P, KD, NJB * P], bf16)
            for k in range(KD):
                nc.sync.dma_start(wkb_sb[:, k], wkb[:, k])

            def wblk(k, j):
                if j < nja:
                    return pk_sb[:, k, WO + j * P:WO + (j + 1) * P]
                return wkb_sb[:, k, (j - nja) * P:(j - nja + 1) * P]

            # Masked low-rank activations, bf16: [sr_p, chunk-band, tok]
            lrm_sb = cpool.tile([P, spc, NTOK], bf16)

            # Warm-up: keep the PE busy (and the HAM clock-gate ramping)
            # while the first input packs are still in flight.  The junk
            # accumulator borrows one accp slot and is released before the
            # last pass-0 group needs its bank.
            junk_sb = cpool.tile([P, P], bf16)
            nc.vector.memset(junk_sb[:], 0.0)
            jp = accp.tile([P, TCH], mybir.dt.float32, tag="acc", name="jp")
            for w in range(JUNK):
                nc.tensor.matmul(
                    jp[:, :P], junk_sb[:], junk_sb[:],
                    start=True, stop=True,
                )

            def stage_a(t, o, k, ps):
                tok = slice(t * TCH, (t + 1) * TCH)
                nc.tensor.matmul(
                    ps[:],
                    pk_sb[:, k, (t * spc + o) * P:(t * spc + o + 1) * P],
                    pk_sb[:, k, XO + t * TCH:XO + (t + 1) * TCH],
                    start=(k == 0),
                    stop=(k == KD - 1),
                )
                if k == KD - 1:
                    # msk[p, o, tok] = (lid[tok] == (o*128+p)//16), host-built
                    nc.vector.tensor_tensor(
                        lrm_sb[:, o, tok],
                        ps[:],
                        msk_sb[:, o, tok],
                        mybir.AluOpType.mult,
                    )

            obs = {}

            def close_group(t, j, po):
                tok = slice(t * TCH, (t + 1) * TCH)
                for o in range(spc):
                    nc.tensor.matmul(
                        po[:],
                        bs_sb[:, t * spc + o, j * P:(j + 1) * P],
                        lrm_sb[:, o, tok],
                        start=False,
                        stop=(o == spc - 1),
                    )
                if j not in obs:
                    obs[j] = wpool.tile([P, NTOK], bf16, tag="ob",
                                        name=f"ob_{j}")
                nc.any.tensor_scalar_add(obs[j][:, tok], po[:],
                                         bia_sb[:, j:j + 1])
                if j == KD - 1:
                    # last f-block: per-chunk DMA so the first half overlaps
                    # the final chunk's close + convert
                    nc.sync.dma_start(out_s[j, :, tok], obs[j][:, tok])
                elif t == NCH - 1:
                    nc.sync.dma_start(out_s[j], obs[j][:])

            run_a = a_in_p0
            if not a_in_p0:
                # Fallback: sequential stage A before the f-block passes.
                for t in range(NCH):
                    for o in range(spc):
                        ps = accp.tile([P, TCH], mybir.dt.float32, tag="acc",
                                       name=f"lr_{t}_{o}")
                        for k in range(KD):
                            stage_a(t, o, k, ps)

            for gi, jg in enumerate(jgs):
                last = gi == len(jgs) - 1
                pos = {}
                lrs = {}
                for t in range(NCH):
                    for j in jg:
                        pos[(t, j)] = accp.tile(
                            [P, TCH], mybir.dt.float32, tag="acc",
                            name=f"po_{t}_{j}")
                    if gi == 0 and run_a:
                        for o in range(spc):
                            lrs[(t, o)] = accp.tile(
                                [P, TCH], mybir.dt.float32, tag="acc",
                                name=f"lr_{t}_{o}")
                if last:
                    # t-major: the first chunk's close/convert/DMA overlaps
                    # the second chunk's matmuls, shortening the tail.
                    for t in range(NCH):
                        for k in range(KD):
                            for j in jg:
                                nc.tensor.matmul(
                                    pos[(t, j)][:],
                                    wblk(k, j),
                                    pk_sb[:, k,
                                          XO + t * TCH:XO + (t + 1) * TCH],
                                    start=(k == 0),
                                    stop=False,
                                )
                        for j in jg:
                            close_group(t, j, pos[(t, j)])
                    continue
                for k in range(KD):
                    for t in range(NCH):
                        if gi == 0 and run_a:
                            for o in range(spc):
                                stage_a(t, o, k, lrs[(t, o)])
                        for j in jg:
                            nc.tensor.matmul(
                                pos[(t, j)][:],
                                wblk(k, j),
                                pk_sb[:, k, XO + t * TCH:XO + (t + 1) * TCH],
                                start=(k == 0),
                                stop=False,
                            )
                for t in range(NCH):
                    for j in jg:
                        close_group(t, j, pos[(t, j)])

    nc.compile()
    return nc


def _get_nc(spc):
    key = (spc, JUNK)
    if key not in _NC_CACHE:
        _NC_CACHE[key] = _build_nc(spc)
    return _NC_CACHE[key]


def kernel(x, adapter_ids, kernel, bias, lora_a, lora_b):
    global LAST_RESULTS, LAST_IN_MAPS, LAST_NC, LAST_NS
    x = np.ascontiguousarray(np.asarray(x, dtype=np.float32))
    adapter_ids = np.asarray(adapter_ids)
    kernel_w = np.asarray(kernel, dtype=np.float32)
    bias = np.asarray(bias, dtype=np.float32)
    lora_a = np.asarray(lora_a, dtype=np.float32)
    lora_b = np.asarray(lora_b, dtype=np.float32)
    ids = adapter_ids.astype(np.int64)

    # Global stable sort by adapter id; each core gets a contiguous run.
    perm = np.argsort(ids, kind="stable")
    ids_s = ids[perm]
    xs_all = x[perm]

    # Per-(core, chunk) adapter band [a0, a0 + 8*spc).
    spans = []
    for cc in range(NCORES * NCH):
        blk = ids_s[cc * TCH:(cc + 1) * TCH]
        spans.append(int(blk.max()) - int(blk.min()) + 1)
    spc = max(1, int(np.ceil(max(spans) / 8)))
    a0s = []
    for cc in range(NCORES * NCH):
        blk = ids_s[cc * TCH:(cc + 1) * TCH]
        a0s.append(min(int(blk.min()), S - 8 * spc) if 8 * spc < S else 0)

    nsl = NCH * spc
    jgs, a_in_p0 = _passes(spc)
    nja = len(jgs[0]) if a_in_p0 else 0
    ACW = nsl * P
    XO = ACW
    WO = ACW + NTOK
    PKW = WO + nja * P
    NJB = KD - nja

    # Replicated weight layouts with contiguous per-partition runs.
    a_cat = lora_a.transpose(1, 0, 2).reshape(D, SR)                  # (D, S*R)
    b_stk = lora_b.reshape(SR, F)                                     # (S*R, F)
    # wk4[k, p, j, fi] = kernel[k*128+p, j*128+fi]
    wk4 = kernel_w.reshape(KD, P, KD, P).astype(BF16)
    wkb_l = np.ascontiguousarray(
        wk4[:, :, nja:, :].reshape(KD, P, NJB * P).transpose(1, 0, 2))
    bia_l = np.ascontiguousarray(bias.reshape(KD, P).T.astype(np.float32))

    # Per-(slab-row, band-slab) local adapter index: (o*128+p)//16
    adiv = (np.arange(spc)[None, :] * P + np.arange(P)[:, None]) // R  # (P, spc)

    in_maps = []
    for c in range(NCORES):
        lo = c * NTOK
        xs = xs_all[lo:lo + NTOK]                                     # (NTOK, D)
        ac_g = np.empty((D, nsl * P), dtype=BF16)
        bs_g = np.empty((nsl, P, F), dtype=BF16)
        msk_l = np.empty((P, spc, NTOK), dtype=BF16)
        for t in range(NCH):
            a0 = a0s[c * NCH + t]
            sr0 = a0 * R
            ac_g[:, (t * spc) * P:(t * spc + spc) * P] = \
                a_cat[:, sr0:sr0 + spc * P].astype(BF16)
            bs_g[t * spc:(t + 1) * spc] = \
                b_stk[sr0:sr0 + spc * P].reshape(spc, P, F).astype(BF16)
            lid = ids_s[lo + t * TCH: lo + (t + 1) * TCH] - a0        # (TCH,)
            msk_l[:, :, t * TCH:(t + 1) * TCH] = \
                (adiv[:, :, None] == lid[None, None, :]).astype(BF16)
        # Pack [A band | x^T | first W f-blocks] per D-slab k.
        pk_l = np.empty((P, KD, PKW), dtype=BF16)
        pk_l[:, :, :ACW] = ac_g.reshape(KD, P, ACW).transpose(1, 0, 2)
        pk_l[:, :, XO:WO] = \
            xs.T.reshape(KD, P, NTOK).transpose(1, 0, 2).astype(BF16)
        pk_l[:, :, WO:] = \
            wk4[:, :, :nja, :].reshape(KD, P, nja * P).transpose(1, 0, 2)
        bs_l = np.ascontiguousarray(bs_g.transpose(1, 0, 2))
        in_maps.append({
            "pk": np.ascontiguousarray(pk_l), "wkb": wkb_l, "bs": bs_l,
            "msk": np.ascontiguousarray(msk_l), "bia": bia_l,
        })

    nc = _get_nc(spc)
    res = run_bass_kernel_spmd(nc, in_maps, core_ids=list(range(NCORES)),
                               trace=TRACE)
    LAST_RESULTS = res
    LAST_IN_MAPS = in_maps
    LAST_NC = nc
    LAST_NS = spc

    out = np.empty((N, F), dtype=np.float32)
    for c in range(NCORES):
        # out_s[j, p, t] holds out^T for f = j*128+p -> reshape to (F, NTOK).
        core_out = res.results[c]["out_s"].reshape(F, NTOK).T
        out[perm[c * NTOK:(c + 1) * NTOK]] = core_out.astype(np.float32)
    return out
